# revision 15
# baseline (speedup 1.0000x reference)
"""Trainium2 Bass kernel for nn_Conv2d_NN (retrieval-knn conv).

Math: x -> concat coords -> pixel_unshuffle(2) -> tokens x2 [136, 1024] per
batch; dist = all-pairs sq-euclidean over tokens; idx = top-9 nearest (incl
self); y = conv1d over gathered neighbors; pixel_shuffle; pointwise conv.
Data-parallel over batch: 4 batches per core on 8 cores.

The wall-clock bottleneck is the axon tunnel (~45 MB/s, half-duplex), so the
structure minimizes bytes moved per call:
- Upload: features packed as 21-bit fixed point in 3 bytes/elem (12.6MB) + f32
  -0.5*sq rows (0.5MB). fp16 features would flip ~0.4% of top-9 neighbor sets
  (rel err 8.5e-2); the 21-bit grid flips ~1e-5 of them. Coords / ones rows /
  folded conv weights are constants, uploaded once and cached device-resident.
- On device the integer features are reassembled exactly (hi*65536 + lo) and
  ranking runs on them in f32: ranking is scale/shift-invariant, so coord/sq
  tails are pre-scaled by SCALE. Gv value matmuls run in fp16 (value error
  ~2e-4); tails (coords + sq + bias, 10 partitions) stay f32.
- Output returned as fp16 [B,128,1024] (8.4MB down), decoded per-shard so host
  unshuffle overlaps the serialized shard downloads; the donated output buffer
  is recycled from the previous call (zeros only on the first call).
- One persistent jit(shard_map) per process — no per-call retracing; the
  feature upload is dispatched async and overlapped with host-side sq prep.
"""
from contextlib import ExitStack

import numpy as np

import concourse.bacc as bacc
import concourse.mybir as mybir
import concourse.tile as tile
from concourse import library_config

B, CIN, H, W = 32, 32, 64, 64
S, K = 2, 9
C1 = (CIN + 2) * S * S          # 136
N = (H // S) * (W // S)         # 1024
NCORES = 8
BPC = B // NCORES               # batches per core
P = 128
NT = N // P                     # 8 n-tiles per batch
NB = N // 512                   # 2 moving-dim blocks

F16 = mybir.dt.float16
F32 = mybir.dt.float32
U8 = mybir.dt.uint8
I8 = mybir.dt.int8
U16 = mybir.dt.uint16
I16 = mybir.dt.int16

# Fixed-point feature encoding (3 bytes/elem): y = x*SCALE + MAGIC has a fixed
# f32 exponent for any |x| < 16, so its low 3 bytes carry the full integer
# mantissa v = y - 2^23; the device ranks on u = v - 2^22 = x * SCALE exactly
# (the 2^22 bias is pre-subtracted from the high byte on the host).
# Ranking is scale-invariant, so coords/sq consts are pre-scaled by SCALE and
# only the fp16 value path multiplies by AINV = 1/SCALE.
SCALE = np.float32(2.0 ** 18)
MAGIC = np.float32(3 * 2 ** 22)
AINV = np.float32(2.0 ** -18)


def _coord_channels() -> np.ndarray:
    """[8, 1024] f32: unshuffled normalized-coordinate channels (x2 rows 128..135)."""
    xg, yg = np.meshgrid(np.arange(H, dtype=np.float64),
                         np.arange(W, dtype=np.float64), indexing="ij")
    nrm = np.maximum(np.sqrt(xg * xg + yg * yg), 1e-12)
    coords = np.stack([xg / nrm, yg / nrm]).astype(np.float32)     # [2, H, W]
    u = coords.reshape(2, H // S, S, W // S, S).transpose(0, 2, 4, 1, 3)
    return u.reshape(8, N)


_COORD8 = _coord_channels()
_SQC = (_COORD8.astype(np.float64) ** 2).sum(axis=0).astype(np.float32)  # [1024]


def _features32(x: np.ndarray) -> np.ndarray:
    """[B, 32, 64, 64] f32 -> [B, 128, 1024] f32 (unshuffle + flatten)."""
    u = x.reshape(B, CIN, H // S, S, W // S, S)
    u = u.transpose(0, 1, 3, 5, 2, 4).reshape(B, P, N)
    return np.ascontiguousarray(u, dtype=np.float32)


def _encode_q22(mains: np.ndarray):
    """[B,128,1024] f32 -> (packed u8 [B,128,3072], u f32 [B,128,1024]).

    packed[..., :2N] = low 16 mantissa bits (le), packed[..., 2N:] = signed
    high byte with the 2^22 bias pre-subtracted, so the device reconstructs
    u = hi*65536 + lo = mains*SCALE rounded to integer."""
    y = mains * SCALE + MAGIC                       # f32 rounding == quantize
    y32 = y.view(np.uint32)
    lo = y32.astype(np.uint16)                      # low 16 bits
    hi = ((y32 >> np.uint32(16)).astype(np.uint8) - np.uint8(64))
    packed = np.concatenate(
        [lo.view(np.uint8).reshape(B, P, 2 * N), hi.reshape(B, P, N)], axis=2)
    u = y - MAGIC                                   # exact: the quantized grid
    return packed, u


def _build_consts(w1, b1, pw_w, pw_b):
    """Fold pixel_shuffle + pointwise conv into per-k weights; build tail mats."""
    w1r = np.asarray(w1, dtype=np.float64).reshape(CIN + 2, S * S, C1, K)
    V = np.einsum("ob,bqck->oqck", np.asarray(pw_w, dtype=np.float64), w1r)
    V = V.reshape(P, C1, K)                                        # [128, 136, 9]
    bfold = np.einsum("ob,bq->oq", np.asarray(pw_w, np.float64),
                      np.asarray(b1, np.float64).reshape(CIN + 2, S * S))
    b_out = (bfold.reshape(P) +
             np.repeat(np.asarray(pw_b, np.float64), S * S))       # [128]

    vt_main = np.zeros((P, K * P), dtype=np.float16)               # rows 0..127 of V_k^T
    vt_tail = np.zeros((48, K * P), dtype=np.float32)              # rows 128..135 (+rep@32)
    for k in range(K):
        vt_main[:, k * P:(k + 1) * P] = V[:, :P, k].T.astype(np.float16)
        # coord rows of tailRC are pre-scaled by SCALE -> compensate with AINV
        vt_tail[0:8, k * P:(k + 1) * P] = (
            V[:, 128:136, k].T * np.float64(AINV)).astype(np.float32)
    vt_tail[9, 0:P] = b_out.astype(np.float32)                     # pairs ones-row (k=0)
    vt_tail[32:48] = vt_tail[0:16]

    # ranking tails: lhsT rows {coords 0-7, ones@8} per 32-group;
    # rhs const rows {coords 0-7, zeros@8 (sq filled per batch), ones@9}.
    # Coord rows carry the SCALE factor to match the integer-scaled mains.
    csc = (_COORD8.astype(np.float64) * np.float64(SCALE)).astype(np.float32)
    tailL = np.zeros((P, N), dtype=np.float32)
    tailRC = np.zeros((P, N), dtype=np.float32)
    for g in range(4):
        tailL[32 * g:32 * g + 8] = csc
        tailL[32 * g + 8] = 1.0
        tailRC[32 * g:32 * g + 8] = csc
        tailRC[32 * g + 9] = 1.0

    diag = np.zeros((P, P), dtype=np.float32)
    np.fill_diagonal(diag, np.float32(-1e30))
    return dict(vt_main=vt_main, vt_tail=vt_tail, tailL=tailL,
                tailRC=tailRC, diag=diag)


def _build_nc():
    nc = bacc.Bacc("TRN2", target_bir_lowering=False, debug=False,
                   num_devices=NCORES)
    packed_d = nc.dram_tensor("packed", [BPC, P, 3 * N], U8, kind="ExternalInput")
    nsq_d = nc.dram_tensor("nsq", [BPC, 4, N], F32, kind="ExternalInput")
    vtm_d = nc.dram_tensor("vt_main", [P, K * P], F16, kind="ExternalInput")
    vtt_d = nc.dram_tensor("vt_tail", [48, K * P], F32, kind="ExternalInput")
    tlc_d = nc.dram_tensor("tailL", [P, N], F32, kind="ExternalInput")
    trc_d = nc.dram_tensor("tailRC", [P, N], F32, kind="ExternalInput")
    diag_d = nc.dram_tensor("diag", [P, P], F32, kind="ExternalInput")
    out_d = nc.dram_tensor("out", [BPC, P, N], F16, kind="ExternalOutput")

    with tile.TileContext(nc) as tc:
        with ExitStack() as ctx:
            consts = ctx.enter_context(tc.tile_pool(name="consts", bufs=1))
            feats = ctx.enter_context(tc.tile_pool(name="feats", bufs=2))
            gvp = ctx.enter_context(tc.tile_pool(name="gvp", bufs=2))
            gop = ctx.enter_context(tc.tile_pool(name="gop", bufs=8))
            small = ctx.enter_context(tc.tile_pool(name="small", bufs=2))
            idxp = ctx.enter_context(tc.tile_pool(name="idxp", bufs=2))
            dram = ctx.enter_context(tc.tile_pool(name="dram", bufs=2, space="DRAM"))
            psg = ctx.enter_context(tc.tile_pool(name="psg", bufs=2, space="PSUM"))
            psr = ctx.enter_context(tc.tile_pool(name="psr", bufs=3, space="PSUM"))

            nc.gpsimd.load_library(library_config.ap_gather)

            # constants
            vtm = consts.tile([P, K * P], F16)
            nc.sync.dma_start(vtm[:], vtm_d.ap())
            vtt = consts.tile([48, K * P], F32)
            nc.sync.dma_start(vtt[:], vtt_d.ap())
            tlc = consts.tile([P, N], F32)
            nc.sync.dma_start(tlc[:], tlc_d.ap())
            trc = consts.tile([P, N], F32)
            nc.sync.dma_start(trc[:], trc_d.ap())
            diag = consts.tile([P, P], F32)
            nc.sync.dma_start(diag[:], diag_d.ap())

            A = mybir.AluOpType
            for b in range(BPC):
                q = feats.tile([P, 3 * N], U8, tag="q")
                nc.sync.dma_start(q[:], packed_d.ap()[b])
                lo_f = feats.tile([P, N], F32, tag="lof")
                nc.vector.tensor_copy(lo_f[:], q[:, 0:2 * N].bitcast(U16))
                main = feats.tile([P, N], F32, tag="main")
                nc.vector.tensor_copy(main[:], q[:, 2 * N:3 * N].bitcast(I8))
                # u = hi*65536 + lo  (= features * SCALE, exact integers)
                nc.vector.scalar_tensor_tensor(main[:], main[:], 65536.0,
                                               lo_f[:], op0=A.mult, op1=A.add)
                m16 = feats.tile([P, N], F16, tag="m16")
                nc.vector.tensor_scalar(m16[:], main[:], float(AINV), None,
                                        op0=A.mult)
                # per-batch rhs tail: const coords/ones + -0.5*sq at rows 8+32g
                tr = feats.tile([P, N], F32, tag="tr")
                nc.vector.tensor_copy(tr[:], trc[:])
                for g in range(4):
                    nc.sync.dma_start(tr[32 * g + 8:32 * g + 9, :],
                                      nsq_d.ap()[b, g:g + 1])

                # ---- ranking r + top8, n-tiles in groups of 3 (packed tails) ----
                idx_dram = dram.tile([16, 512], U16, tag="idxd")
                for grp in ((0, 1, 2), (3, 4, 5), (6, 7)):
                    rpss = []
                    for nt in grp:
                        ms = slice(nt * P, (nt + 1) * P)
                        rps = psr.tile([P, N], F32, tag="r")
                        rpss.append(rps)
                        for nb in range(NB):
                            cs = slice(nb * 512, (nb + 1) * 512)
                            nc.tensor.matmul(rps[:, cs], main[:, ms], main[:, cs],
                                             start=True, stop=False)
                    # 10-row tail matmuls packed into distinct PE row-groups
                    for nb in range(NB):
                        cs = slice(nb * 512, (nb + 1) * 512)
                        for i, nt in enumerate(grp):
                            ms = slice(nt * P, (nt + 1) * P)
                            nc.tensor.matmul(rpss[i][:, cs],
                                             tlc[32 * i:32 * i + 10, ms],
                                             tr[32 * i:32 * i + 10, cs],
                                             start=False, stop=True,
                                             tile_position=(32 * i, 0))
                    for i, nt in enumerate(grp):
                        ms = slice(nt * P, (nt + 1) * P)
                        rps = rpss[i]
                        nc.vector.tensor_add(rps[:, ms], rps[:, ms], diag[:])
                        mx = small.tile([P, 8], F32, tag="mx")
                        mi = small.tile([P, 8], U16, tag="mi")
                        nc.vector.max(out=mx[:], in_=rps[:])
                        nc.vector.max_index(out=mi[:], in_max=mx[:], in_values=rps[:])
                        # scatter chunk nt into the wrap layout:
                        # dst[lo, j*64 + nt*8 + hi] = mi[hi*16+lo, j]
                        dst = idx_dram[:].rearrange(
                            "lo (j gg h) -> gg h lo j", j=8, gg=8, h=8)[nt]
                        nc.scalar.dma_start(dst, mi[:])

                # ---- replicate wrap to all 8 16-partition groups (contiguous reads)
                wrap = idxp.tile([P, 512], U16, tag="wrap")
                for g in range(8):
                    nc.sync.dma_start(wrap[g * 16:(g + 1) * 16, :], idx_dram[:])

                # ---- Gv_k = V_k @ x2 (+bias via ones row); tails k-paired
                gvcat = gvp.tile([P, K * N], F32, tag="gvcat")
                for kp in range(5):
                    ks = (2 * kp, 2 * kp + 1) if kp < 4 else (8,)
                    for nb in range(NB):
                        cs = slice(nb * 512, (nb + 1) * 512)
                        gpss = []
                        for k in ks:
                            gps = psg.tile([P, 512], F32, tag="gv")
                            gpss.append(gps)
                            nc.tensor.matmul(gps[:],
                                             vtm[:, k * P:(k + 1) * P],
                                             m16[:, cs], start=True, stop=False)
                        for i, k in enumerate(ks):
                            nc.tensor.matmul(gpss[i][:],
                                             vtt[32 * i:32 * i + 10,
                                                 k * P:(k + 1) * P],
                                             tr[32 * i:32 * i + 10, cs],
                                             start=False, stop=True,
                                             tile_position=(32 * i, 0))
                        for i, k in enumerate(ks):
                            nc.scalar.copy(
                                gvcat[:, k * N + nb * 512:k * N + (nb + 1) * 512],
                                gpss[i][:])

                # ---- per-j gathers + accumulate; final written as fp16
                gjs = []
                for j in range(8):
                    gj = gop.tile([P, N], F32, tag="gout")
                    gjs.append(gj)
                    nc.gpsimd.ap_gather(
                        gj[:], gvcat[:, (j + 1) * N:(j + 2) * N],
                        wrap[:, j * 64:(j + 1) * 64].bitcast(I16),
                        channels=P, num_elems=N, d=1, num_idxs=N)
                A = mybir.AluOpType
                for a, c in ((0, 1), (2, 3), (4, 5), (6, 7), (0, 2), (4, 6), (0, 4)):
                    nc.vector.scalar_tensor_tensor(gjs[a][:], gjs[a][:], 1.0,
                                                   gjs[c][:], op0=A.mult, op1=A.add)
                fin = small.tile([P, N], F16, tag="fin")
                nc.vector.scalar_tensor_tensor(fin[:], gjs[0][:], 1.0,
                                               gvcat[:, 0:N], op0=A.mult, op1=A.add)
                nc.sync.dma_start(out_d.ap()[b], fin[:])

    nc.finalize()
    return nc


class _Runner:
    """Persistent jit(shard_map) around the bass custom call, with
    device-resident constants and a recycled donated output buffer."""

    def __init__(self, nc):
        import jax
        from jax.sharding import Mesh, PartitionSpec, NamedSharding
        from jax.experimental.shard_map import shard_map
        from concourse.bass2jax import (
            _bass_exec_p, partition_id_tensor, install_neuronx_cc_hook)

        install_neuronx_cc_hook()
        self.jax = jax
        self.nc = nc
        partition_name = (nc.partition_id_tensor.name
                          if nc.partition_id_tensor else None)
        in_names, out_names, out_avals = [], [], []
        for alloc in nc.m.functions[0].allocations:
            if not isinstance(alloc, mybir.MemoryLocationSet):
                continue
            name = alloc.memorylocations[0].name
            if alloc.kind == "ExternalInput":
                if name != partition_name:
                    in_names.append(name)
            elif alloc.kind == "ExternalOutput":
                out_names.append(name)
                out_avals.append(jax.core.ShapedArray(
                    tuple(alloc.tensor_shape), mybir.dt.np(alloc.dtype)))
        assert out_names == ["out"], out_names
        self.in_names = in_names
        all_names = list(in_names) + list(out_names)
        if partition_name is not None:
            all_names.append(partition_name)
        n_params = len(in_names)
        n_outs = len(out_names)

        def _body(*args):
            operands = list(args)
            if partition_name is not None:
                operands.append(partition_id_tensor())
            outs = _bass_exec_p.bind(
                *operands,
                out_avals=tuple(out_avals),
                in_names=tuple(all_names),
                out_names=tuple(out_names),
                lowering_input_output_aliases=(),
                sim_require_finite=True,
                sim_require_nnan=True,
                nc=nc,
            )
            return tuple(outs)

        devices = jax.devices()[:NCORES]
        assert len(devices) == NCORES
        self.mesh = Mesh(np.asarray(devices), ("core",))
        self.sharding = NamedSharding(self.mesh, PartitionSpec("core"))
        in_specs = (PartitionSpec("core"),) * (n_params + n_outs)
        out_specs = (PartitionSpec("core"),) * n_outs
        self.sharded = jax.jit(
            shard_map(_body, mesh=self.mesh, in_specs=in_specs,
                      out_specs=out_specs, check_rep=False),
            donate_argnums=(n_params,), keep_unused=True)
        self.const_dev = {}
        self.out_buf = None

    def upload_consts(self, consts: dict):
        """Device-put the replicated constants once per process."""
        for name, arr in consts.items():
            glob = np.concatenate([arr] * NCORES, axis=0)
            self.const_dev[name] = self.jax.device_put(glob, self.sharding)

    def put(self, arr: np.ndarray):
        """Async sharded upload (does not block)."""
        return self.jax.device_put(arr, self.sharding)

    def __call__(self, packed, nsq, sink):
        """Run one batch; stream per-shard results into sink(core, host_slab)
        so host-side decode overlaps the serialized shard downloads."""
        args = []
        for name in self.in_names:
            if name == "packed":
                args.append(packed)
            elif name == "nsq":
                args.append(nsq)
            else:
                args.append(self.const_dev[name])
        donate = self.out_buf
        if donate is None or (hasattr(donate, "is_deleted")
                              and donate.is_deleted()):
            donate = np.zeros((B, P, N), np.float16)
        (out,) = self.sharded(*args, donate)
        self.out_buf = out
        shards = sorted(out.addressable_shards, key=lambda s: s.index[0].start)
        for sh in shards:
            sh.data.copy_to_host_async()
        for c, sh in enumerate(shards):
            sink(c, np.asarray(sh.data))


_CACHE = {}


def kernel(x, w1, b1, pw_w, pw_b):
    x = np.asarray(x, dtype=np.float32)
    mains = _features32(x)                                          # [32,128,1024] f32
    packed, u = _encode_q22(mains)                                  # 12.6MB + grid vals

    if "runner" not in _CACHE:
        _CACHE["nc"] = _build_nc()
        _CACHE["runner"] = _Runner(_CACHE["nc"])
    runner = _CACHE["runner"]

    # start the big upload immediately; overlap host-side prep with it
    dpacked = runner.put(packed)

    import hashlib
    h = hashlib.blake2b(digest_size=16)
    for a in (w1, b1, pw_w, pw_b):
        h.update(np.ascontiguousarray(np.asarray(a)).tobytes())
    wkey = h.digest()
    if _CACHE.get("wkey") != wkey:
        consts = _build_consts(w1, b1, pw_w, pw_b)
        runner.upload_consts(consts)
        _CACHE["wkey"] = wkey

    # sq in the same SCALE^2-space as the on-device integer features
    sq_main = np.einsum("bcn,bcn->bn", u, u)                        # [32,1024]
    sqc = (_SQC.astype(np.float64) * (np.float64(SCALE) ** 2)).astype(np.float32)
    nsq = np.broadcast_to(
        (-0.5 * (sq_main + sqc[None]))[:, None, :], (B, 4, N))
    nsq = np.ascontiguousarray(nsq, dtype=np.float32)               # [32,4,1024]

    out = np.empty((B, CIN, H, W), dtype=np.float32)
    # [b, cout, hs, sy, ws, sx] view: assigning the [cout, sy, sx, hs, ws]
    # device layout into it performs shuffle + f16->f32 in one pass
    outv = out.reshape(B, CIN, H // S, S, W // S, S).transpose(0, 1, 3, 5, 2, 4)

    def sink(c, slab):                                              # [BPC,128,1024] f16
        outv[c * BPC:(c + 1) * BPC] = slab.reshape(
            BPC, CIN, S, S, H // S, W // S)

    runner(dpacked, nsq, sink)
    return out
